# revision 50
# baseline (speedup 1.0000x reference)
"""Bass/Tile Trainium2 kernel for nn_Attention_9929964388721.

Module: 4-head spatial attention over [b=4, c=256, 64, 64] images.
  qkv = w_qkv @ x  (1x1 conv), split q/k/v with heads=4, dim_head=32,
  q,k l2-normalized over dim_head, sim = 10 * q^T k  (n=4096 tokens),
  attn = softmax(sim), out = attn @ v, y = w_out @ out + b_out.

Sharding (8 cores): core c handles batch b = c//2 and heads {2*(c%2), 2*(c%2)+1}.
Each core computes its partial y contribution [256, 4096]; the host sums the
two partials per batch and adds the bias.

v2 pipeline (exp split across ACT and DVE, col-packed PV):
  - qs is pre-scaled by 10*1024*log2(e)/|q| so the sim matmul emits
    t = 1024*log2(e)*sim directly. ACT tiles: exp via activation
    (scale=ln2/1024). DVE tiles: custom fast-exp op emits fp16 bit patterns
    (round-split + quadratic mantissa fill + exact 2x negative-branch rule,
    max rel err ~2.3e-3) via int16 output convert at 1 elem/cycle.
  - PV: per j-tile an M=32 av matmul (col group j%4, tile_position) plus an
    M=1 rowsum matmul (ones/1024 stationary, v pre-scaled by 2^-10 so
    av/rowsum scales cancel). 4 col groups stream concurrently.
  - Per chunk: rowsum partials folded via a sel matmul (rows 0/32/64/96),
    RECIPROCAL_APPROX_FAST on DVE, oT = av * (1/S) into fp16.
  - Tail: y = wo4^T @ oT per chunk/half, K=128 matmuls accumulate both units
    and the 4 col-group partials in one pass.
"""

import sys

sys.path.insert(0, "/opt/trn_rl_repo")

from contextlib import ExitStack

import numpy as np

import concourse.mybir as mybir
import concourse.tile as tile
from concourse import bacc
from concourse.bass_utils import run_bass_kernel_spmd

HEADS = 4
DIM_HEAD = 32
B, C, H, W = 4, 256, 64, 64
N = H * W                  # 4096 tokens
HIDDEN = HEADS * DIM_HEAD  # 128
NCORES = 8
UNITS = 2                  # (batch, head) pairs per core
CHUNK = 512                # i-chunk width
NCHUNK = N // CHUNK        # 8
JT = N // 128              # 32 j-tiles of 128
F32 = mybir.dt.float32
F16 = mybir.dt.float16
I16 = mybir.dt.int16

A_PRE = 1024.0 * float(np.log2(np.e))     # folded into rq
LNPRE = float(np.log(10.0 * A_PRE))       # ln(10 * A_PRE) for the rq chain
LN2_1024 = float(np.log(2.0) / 1024.0)    # ACT exp scale
# floor-mode fast-exp: v = t + C1 lands in [2^23, 2^24) (exp byte 0x96);
# AND with MASK clears the low 10 mantissa bits -> 1024-aligned floor.
EXP_C1 = 12598272.0                       # 3*2^22 + 15360
EXP_MASK = float(np.uint32(0x4B7FFC00).view(np.float32))
EXP_B1 = 0.6659735417740476               # mantissa-fill poly on [0, 1024)
EXP_B2 = 0.0003221902763099779
VS = 2.0**-10                             # v / rowsum scale (cancels exactly)

# ---- custom DVE fast-exp op registration ---------------------------------


def _register_fast_exp():
    from concourse import dve_ops as dops
    from concourse.dve_spec import (
        C0, C1, C2, C3, AluOp, Bin, Spec, Src0, lower,
        _spill_c3_to_src1,
    )
    from concourse.dve_uop import DveOpSpec

    name = "FAST_EXP2_BITS_ANT"
    for op in dops.OPS:
        if op.name == name:
            return op

    # body (7 ALU ops): v = t + C1; rA = v & C2(mask); f = v - rA;
    # pm = f*(f*C3 + C0); out = pm + rA.  Written as int32; the low 16 bits
    # are the fp16 pattern because C1's 1024-aligned base is 192*65536+15360.
    v = Src0 + C1
    rA = Bin(AluOp.BITWISE_AND, v, C2)
    f = v - rA
    pm = f * (f * C3 + C0)
    body = pm + rA

    def _ref(in0, in1, s0, s1, imm2):
        t = in0.astype(np.float32)
        vv = (t + np.float32(s1)).astype(np.float32)
        mask = np.float32(imm2).view(np.uint32)
        ra = (vv.view(np.uint32) & mask).view(np.float32)
        ff = (vv - ra).astype(np.float32)
        b2 = in1.reshape(in1.shape[0], 1).astype(np.float32)
        p = ff * (ff * b2 + np.float32(s0))
        return p + ra

    spec = Spec(body=_spill_c3_to_src1(body), reference=_ref)
    row = dops._CUSTOM_DVE_ROW_BASE + len(dops.OPS)
    shas = {}
    for ver in ("v3", "v4"):
        tmp = DveOpSpec(name=name, opcode=row, uops=lower(spec, ver=ver),
                        rd1_en=True)
        shas[ver] = tmp.sha(ver)
    op = dops.DveOp(name, spec, subdim=False, uops_sha=shas)
    dops.OPS.append(op)
    dops._SUB_OPCODE_FOR_NAME[name] = row
    dops.CUSTOM_DVE_SPECS[name] = spec
    return op


FAST_EXP_OP = _register_fast_exp()


def _p2_groups(start_slot):
    """Group sizes for one chunk: rotate three 2-bank slots (A/B/C) so the
    sim matmuls run up to three groups ahead of the exp consumers (keeps the
    PE saturated and HAM-warm). Returns list of (size, slot)."""
    out = []
    rem, slot = JT, start_slot
    while rem:
        g = min(rem, 2)
        out.append((g, slot))
        rem -= g
        slot = (slot + 1) % 3
    return out


def _build():
    from concourse.dve_ops import RECIPROCAL_APPROX_FAST, RECIP_APPROX_FAST_CONSTS

    nc = bacc.Bacc("TRN2", target_bir_lowering=False, debug=False,
                   num_devices=NCORES)

    # ---- DRAM I/O ----
    x_in = nc.dram_tensor("x_in", [C, N], F16, kind="ExternalInput").ap()
    wqT = nc.dram_tensor("wqT", [UNITS, C, 128], F16, kind="ExternalInput").ap()
    wkT = nc.dram_tensor("wkT", [UNITS, C, DIM_HEAD], F16,
                         kind="ExternalInput").ap()
    wvT = nc.dram_tensor("wvT", [C, 2 * DIM_HEAD], F16,
                         kind="ExternalInput").ap()
    wo4 = nc.dram_tensor("wo4", [UNITS, 97, 2, 128], F16,
                         kind="ExternalInput").ap()
    ones4 = nc.dram_tensor("ones4", [128, 4], F16, kind="ExternalInput").ap()
    y_out = nc.dram_tensor("y_out", [C, N], F32, kind="ExternalOutput").ap()

    with ExitStack() as top:
        tc = top.enter_context(tile.TileContext(nc))
        persist = top.enter_context(tc.tile_pool(name="persist", bufs=1))
        p12 = top.enter_context(ExitStack())
        chains = p12.enter_context(tc.tile_pool(name="chains", bufs=1))
        dram = top.enter_context(tc.tile_pool(name="dram", bufs=1, space="DRAM"))

        qs = [persist.tile([128, N], F16, name=f"qs{u}", tag=f"qs{u}")
              for u in range(UNITS)]
        kg = [persist.tile([128, N // 4], F16, name=f"kg{u}", tag=f"kg{u}")
              for u in range(UNITS)]
        kg_raw = [persist.tile([128, N // 4], F32, name=f"kg_raw{u}",
                               tag=f"kg_raw{u}") for u in range(UNITS)]
        # vt97: cols 0-31 = v dims, col 32 = ones (rowsum), 33-96 zero
        # padding so the PV matmul is full-array (col mask 0xF) and feeds the
        # HAM activity monitor -- masked matmuls leave the PE clock-throttled.
        vt = [persist.tile([128, JT, 97], F16, name=f"vt{u}",
                           tag=f"vt{u}") for u in range(UNITS)]
        oT = [persist.tile([97, N], F16, name=f"oT{u}", tag=f"oT{u}")
              for u in range(UNITS)]
        w_o = persist.tile([97, UNITS, 2, 128], F16, name="w_o", tag="w_o")
        for u in range(UNITS):
            nc.sync.dma_start(out=w_o[:, u, :, :], in_=wo4[u, :, :, :])
        # C3 spill value for the fast-exp op
        b2t = persist.tile([128, 1], F32, name="b2t", tag="b2t")
        nc.vector.memset(b2t[:, :], EXP_B2)

        # =========================== P1: projections =======================
        # PSUM pool creation order fixes bank addresses:
        #   pq 0-1, pssq 2-3, pk 4-5, pv 6-7.
        with ExitStack() as p1:
            wpool = p1.enter_context(tc.tile_pool(name="wpool", bufs=1))
            sc = p1.enter_context(tc.tile_pool(name="p1scratch", bufs=2))
            pq = p1.enter_context(tc.tile_pool(name="pq", bufs=2, space="PSUM"))
            pssq = p1.enter_context(tc.tile_pool(name="pssq", bufs=2,
                                                 space="PSUM"))
            pk = p1.enter_context(tc.tile_pool(name="pk", bufs=2, space="PSUM"))
            pv = p1.enter_context(tc.tile_pool(name="pv", bufs=2, space="PSUM"))

            w_q = wpool.tile([128, 2, UNITS, 128], F16, name="w_q", tag="w_q")
            w_k = wpool.tile([128, 2, UNITS, DIM_HEAD], F16, name="w_k",
                             tag="w_k")
            w_v = wpool.tile([128, 2, 2 * DIM_HEAD], F16, name="w_v", tag="w_v")
            for dst, srct in ((w_q, wqT), (w_k, wkT)):
                srcv = srct.rearrange("u (kt p) m -> p kt u m", p=128)
                for kt in range(2):
                    for u in range(UNITS):
                        nc.sync.dma_start(out=dst[:, kt, u, :],
                                          in_=srcv[:, kt, u, :])
            nc.sync.dma_start(out=w_v[:, :, :],
                              in_=wvT.rearrange("(kt p) m -> p kt m", p=128))
            o4 = wpool.tile([128, 4], F16, name="o4", tag="o4")
            nc.sync.dma_start(out=o4[:, :], in_=ones4)

            x_sb = wpool.tile([128, 2, N], F16, name="x_sb", tag="x_sb")
            x_view = x_in.rearrange("(kt p) n -> p kt n", p=128)
            for ch in range(NCHUNK):
                for kt in range(2):
                    nc.sync.dma_start(
                        out=x_sb[:, kt, ch * CHUNK:(ch + 1) * CHUNK],
                        in_=x_view[:, kt, ch * CHUNK:(ch + 1) * CHUNK])

            # --- vT projection (x stationary, units merged, N=64) ---
            for u in range(UNITS):
                nc.vector.memset(vt[u][:, :, :], 0.0)
                nc.vector.memset(vt[u][:, :, DIM_HEAD:DIM_HEAD + 1], 1.0)
            for jt in range(JT):
                ps = pv.tile([128, 2 * DIM_HEAD], F32, name="psv", tag="psv")
                for kt in range(2):
                    nc.tensor.matmul(
                        ps[:, :],
                        x_sb[:, kt, jt * 128:(jt + 1) * 128],
                        w_v[:, kt, :],
                        start=(kt == 0), stop=(kt == 1))
                for u in range(UNITS):
                    nc.vector.tensor_copy(
                        vt[u][:, jt, 0:DIM_HEAD],
                        ps[:, u * DIM_HEAD:(u + 1) * DIM_HEAD])

            q_rep = [chains.tile([128, N], F32, name=f"q_rep{u}",
                                 tag=f"q_rep{u}") for u in range(UNITS)]
            # rr_d[u, 0] = rq values (r, ch, jj); rr_d[u, 1] = rk (r, c, jj)
            rr_d = dram.tile([UNITS, 2, N], F32, name="rr_d", tag="rr_d")
            lnpre_t = chains.tile([4, 1], F32, name="lnpre_t", tag="lnpre_t")
            nc.vector.memset(lnpre_t[:, :], LNPRE)

            sstq = [chains.tile([4, NCHUNK, 128], F32, name=f"sstq{u}",
                             tag=f"sstq{u}") for u in range(UNITS)]
            sstk = [chains.tile([4, NCHUNK, 128], F32, name=f"sstk{u}",
                             tag=f"sstk{u}") for u in range(UNITS)]

            # ---- per-unit: phase A (PSUM work) then phase B (norm chain) ----
            for u in range(UNITS):
                # q replicated projection (for the sim matmuls)
                for ch in range(NCHUNK):
                    ps = pq.tile([128, CHUNK], F32, name="psq", tag="psq")
                    for kt in range(2):
                        nc.tensor.matmul(
                            ps[:, :],
                            w_q[:, kt, u, :],
                            x_sb[:, kt, ch * CHUNK:(ch + 1) * CHUNK],
                            start=(kt == 0), stop=(kt == 1))
                    nc.scalar.copy(
                        q_rep[u][:, ch * CHUNK:(ch + 1) * CHUNK], ps[:, :])

                # grouped q projection, only for its sum-of-squares
                for h in range(2):
                    ps = pk.tile([128, CHUNK], F32, name="psk", tag="psk")
                    for r in range(4):
                        for kt in range(2):
                            xv = x_sb[:, kt, :].rearrange(
                                "p (blk cc jj) -> p blk cc jj", cc=4, jj=128)
                            nc.tensor.matmul(
                                ps[32 * r:32 * r + 32, :],
                                w_q[:, kt, u, 0:DIM_HEAD],
                                xv[:, 4 * h:4 * h + 4, r, :],
                                start=(kt == 0), stop=(kt == 1),
                                tile_position=(0, 32 * r))
                    sq = sc.tile([128, CHUNK], F16, name="sq", tag="sq")
                    qg_sb = sc.tile([128, CHUNK], F32, name="qg_sb",
                                    tag="qg_sb")
                    nc.vector.tensor_copy(qg_sb[:, :], ps[:, :])
                    nc.vector.tensor_mul(sq[:, :], qg_sb[:, :], qg_sb[:, :])
                    ps2 = pssq.tile([4, CHUNK], F32, name="psssq", tag="psssq")
                    nc.tensor.matmul(ps2[:, :], o4[:, :], sq[:, :],
                                     start=True, stop=True)
                    nc.vector.tensor_copy(
                        sstq[u][:, 4 * h:4 * h + 4, :],
                        ps2[:, :].rearrange("r (cc jj) -> r cc jj", jj=128))

                # k grouped projection + ssq
                for h in range(2):
                    ps = pk.tile([128, CHUNK], F32, name="psk", tag="psk")
                    for r in range(4):
                        for kt in range(2):
                            xv = x_sb[:, kt, :].rearrange(
                                "p (blk cc jj) -> p blk cc jj", cc=4, jj=128)
                            nc.tensor.matmul(
                                ps[32 * r:32 * r + 32, :],
                                w_k[:, kt, u, :],
                                xv[:, 4 * h:4 * h + 4, r, :],
                                start=(kt == 0), stop=(kt == 1),
                                tile_position=(0, 32 * r))
                    nc.vector.tensor_copy(
                        kg_raw[u][:, h * CHUNK:(h + 1) * CHUNK], ps[:, :])
                    sq = sc.tile([128, CHUNK], F16, name="sq", tag="sq")
                    kr = kg_raw[u][:, h * CHUNK:(h + 1) * CHUNK]
                    nc.vector.tensor_mul(sq[:, :], kr, kr)
                    ps2 = pssq.tile([4, CHUNK], F32, name="psssq", tag="psssq")
                    nc.tensor.matmul(ps2[:, :], o4[:, :], sq[:, :],
                                     start=True, stop=True)
                    nc.vector.tensor_copy(
                        sstk[u][:, 4 * h:4 * h + 4, :],
                        ps2[:, :].rearrange("r (cc jj) -> r cc jj", jj=128))

                nc.scalar.activation(sstq[u][:, :, :], sstq[u][:, :, :],
                                     mybir.ActivationFunctionType.Ln)
                nc.scalar.activation(sstk[u][:, :, :], sstk[u][:, :, :],
                                     mybir.ActivationFunctionType.Ln)
                # rq = 10*A_PRE/|q| = exp(-0.5*ln(ssq) + LNPRE);  rk = 1/|k|
                nc.scalar.activation(sstq[u][:, :, :], sstq[u][:, :, :],
                                     mybir.ActivationFunctionType.Exp,
                                     bias=lnpre_t[:, :], scale=-0.5)
                nc.scalar.activation(sstk[u][:, :, :], sstk[u][:, :, :],
                                     mybir.ActivationFunctionType.Exp,
                                     bias=0.0, scale=-0.5)
                for a, sst in ((0, sstq[u]), (1, sstk[u])):
                    nc.gpsimd.dma_start(
                        out=rr_d[u, a, :].rearrange("(c r jj) -> r c jj",
                                                    r=4, jj=128),
                        in_=sst[:, :, :])

                rqb = chains.tile([128, N], F32, name="rqb", tag="rqb")
                if u == 0:
                    # broadcast on GPSIMD, quartered for pipelining
                    rq_row = chains.tile([1, N], F32, name="rq_row",
                                         tag="rq_row")
                    nc.sync.dma_start(out=rq_row[:, :],
                                      in_=rr_d[u, 0, :].unsqueeze(0))
                    for hh in range(4):
                        hs = slice(hh * (N // 4), (hh + 1) * (N // 4))
                        nc.gpsimd.partition_broadcast(rqb[:, hs],
                                                      rq_row[:, hs])
                        nc.vector.tensor_mul(qs[u][:, hs], q_rep[u][:, hs],
                                             rqb[:, hs])
                else:
                    # broadcast-DMAs (overlap the running mainloop)
                    for ch in range(NCHUNK):
                        eng = nc.sync if ch % 2 == 0 else nc.gpsimd
                        eng.dma_start(
                            out=rqb[:, ch * CHUNK:(ch + 1) * CHUNK],
                            in_=rr_d[u, 0, ch * CHUNK:(ch + 1) * CHUNK]
                            .partition_broadcast(128))
                    for hh in range(4):
                        hs = slice(hh * (N // 4), (hh + 1) * (N // 4))
                        nc.vector.tensor_mul(qs[u][:, hs], q_rep[u][:, hs],
                                             rqb[:, hs])
                rkb = chains.tile([128, N // 4], F32, name="rkb", tag="rkb")
                rkv = rr_d[u, 1, :].rearrange("(c r jj) -> r c jj", r=4,
                                              jj=128)
                for r in range(4):
                    eng = nc.sync if r % 2 == 0 else nc.gpsimd
                    eng.dma_start(
                        out=rkb[32 * r:32 * r + 32, :].rearrange(
                            "p (c jj) -> p c jj", jj=128),
                        in_=rkv[r, :, :].partition_broadcast(32))
                for hh in range(2):
                    hs = slice(hh * (N // 8), (hh + 1) * (N // 8))
                    nc.vector.tensor_mul(kg[u][:, hs], kg_raw[u][:, hs],
                                         rkb[:, hs])

        # =========================== P2: attention =========================
        # PSUM banks: stA 0-1, stB 2-3, stC 4-5, av 6-7 (double buffered).
        with ExitStack() as p2:
            pstA = p2.enter_context(tc.tile_pool(name="pstA", bufs=1,
                                                 space="PSUM"))
            pstB = p2.enter_context(tc.tile_pool(name="pstB", bufs=1,
                                                 space="PSUM"))
            pstC = p2.enter_context(tc.tile_pool(name="pstC", bufs=1,
                                                 space="PSUM"))
            pav = p2.enter_context(tc.tile_pool(name="pav", bufs=2,
                                                space="PSUM"))
            ptp = p2.enter_context(tc.tile_pool(name="ptp", bufs=4))
            sc2 = p2.enter_context(tc.tile_pool(name="p2scratch", bufs=2))

            rconst = RECIP_APPROX_FAST_CONSTS
            rcp_d = dram.tile([UNITS, NCHUNK, 2, CHUNK], F32, name="rcp_d",
                              tag="rcp_d")

            # ---- flat global pipeline over all (unit, chunk, group) --------
            # No per-chunk barriers: sims stream continuously; exp lags one
            # group; av lags three; each chunk's epilogue interleaves into the
            # next chunk's emission so no engine ever drains.
            sched = []  # (u, ch, jt0, g, slot, eng)
            slot = 0
            for u in range(UNITS):
                for ch in range(NCHUNK):
                    jt0 = 0
                    for gidx in range(16):
                        eng = "ACT" if (gidx % 2 == 0 or gidx == 15) else "DVE"
                        sched.append((u, ch, jt0, 2, slot, eng))
                        slot = (slot + 1) % 3
                        jt0 += 2

            av_tiles = {}

            def get_av(u, ch):
                key = (u, ch)
                if key not in av_tiles:
                    t = pav.tile([97, CHUNK], F32, name="av", tag="av")
                    av_tiles[key] = t
                    if u == 0 and ch < 2:
                        # one-time per buffer: rows 33-63 are never written
                        # by the av matmuls but flow through the oT mul
                        # (killed later by zero w_o rows) — keep them finite.
                        nc.vector.memset(t[DIM_HEAD:64, :], 0.0)
                return av_tiles[key]

            def emit_exp(rec):
                st, pt, eng, g = rec[0], rec[1], rec[2], rec[5]
                stf = st[:, 0:g, :].rearrange("p g f -> p (g f)")
                ptf = pt[:, 0:g, :].rearrange("p g f -> p (g f)")
                if eng == "ACT":
                    nc.scalar.activation(
                        ptf, stf, mybir.ActivationFunctionType.Exp,
                        bias=0.0, scale=LN2_1024)
                else:
                    nc.vector._custom_dve(
                        FAST_EXP_OP, out=ptf, in0=stf, in1=b2t[:, :],
                        s0=EXP_B1, s1=EXP_C1, imm2=EXP_MASK)

            def emit_avrs(rec):
                st, pt, eng, u, ch, g, jt0 = (rec[0], rec[1], rec[2],
                                              rec[3], rec[4], rec[5], rec[6])
                av = get_av(u, ch)
                for s in range(g):
                    j = jt0 + s
                    if eng == "ACT":
                        rhs = pt[:, s, :]
                    else:
                        rhs = pt[:, s, :].bitcast(F16).rearrange(
                            "p (n two) -> p n two", two=2)[:, :, 0]
                    if j % 4 == 3:
                        # full-array feeder: col mask 0xF keeps HAM warm;
                        # cols 33-96 of vt are zeros so rows 33-96 accumulate 0
                        nc.tensor.matmul(
                            av[:, :], vt[u][:, j, :], rhs,
                            start=False, stop=(j == JT - 1),
                            skip_group_check=True)
                    else:
                        c2 = 64 * (j % 2)
                        nc.tensor.matmul(
                            av[c2:c2 + 33, :], vt[u][:, j, 0:33], rhs,
                            start=(j < 2), stop=False,
                            tile_position=(0, c2), skip_group_check=True)

            def emit_epilogue_head(u, ch):
                # rowsum rows (32, 96) -> SBUF -> DRAM -> 97-lane broadcasts
                # -> GPSIMD add. The recip+scale (fin) is deferred further.
                av = av_tiles[(u, ch)]
                rsr = sc2.tile([97, CHUNK], F32, name="rsr", tag="rsr")
                nc.scalar.copy(rsr[:, :], av[:, :])
                nc.gpsimd.dma_start(
                    out=rcp_d[u, ch, 0, :].unsqueeze(0),
                    in_=rsr[DIM_HEAD:DIM_HEAD + 1, :])
                nc.sync.dma_start(
                    out=rcp_d[u, ch, 1, :].unsqueeze(0),
                    in_=rsr[96:97, :])
                rba = sc2.tile([97, CHUNK], F32, name="rba", tag="rba")
                rbb = sc2.tile([97, CHUNK], F32, name="rbb", tag="rbb")
                nc.sync.dma_start(
                    out=rba[:, :],
                    in_=rcp_d[u, ch, 0, :].partition_broadcast(97))
                nc.gpsimd.dma_start(
                    out=rbb[:, :],
                    in_=rcp_d[u, ch, 1, :].partition_broadcast(97))
                rbs = sc2.tile([97, CHUNK], F32, name="rbs", tag="rbs")
                nc.gpsimd.tensor_add(rbs[:, :], rba[:, :], rbb[:, :])
                del av_tiles[(u, ch)]
                return (u, ch, rsr, rbs)

            def emit_fin(state):
                u, ch, rsr, rbs = state
                i0 = ch * CHUNK
                rrb = sc2.tile([97, CHUNK], F32, name="rrb", tag="rrb")
                nc.vector._custom_dve(
                    RECIPROCAL_APPROX_FAST, out=rrb[:, :], in0=rbs[:, :],
                    s0=rconst["s0"], s1=rconst["s1"], imm2=rconst["imm2"])
                nc.vector.tensor_mul(oT[u][:, i0:i0 + CHUNK],
                                     rsr[:, :], rrb[:, :])

            EXP_LAG, AV_LAG, FIN_LAG = 1, 3, 10
            recs = []
            epi_states = []
            fin_states = []
            NTOT = len(sched)
            def emit_sims(k):
                u, ch, jt0, g, slot, eng = sched[k]
                pool = (pstA, pstB, pstC)[slot]
                st = pool.tile([128, 2, CHUNK], F32, name=f"st{slot}",
                               tag=f"st{slot}")
                if eng == "ACT":
                    pt = ptp.tile([128, 2, CHUNK], F16, name="ptA",
                                  tag="ptA")
                else:
                    pt = ptp.tile([128, 2, CHUNK], mybir.dt.int32,
                                  name="ptD", tag="ptD")
                i0 = ch * CHUNK
                for s in range(g):
                    j = jt0 + s
                    r = j % 4
                    t = j // 4
                    nc.tensor.matmul(
                        st[:, s, :],
                        kg[u][32 * r:32 * r + 32, t * 128:(t + 1) * 128],
                        qs[u][32 * r:32 * r + 32, i0:i0 + CHUNK],
                        start=True, stop=True,
                        tile_position=(32 * r, 0))
                recs.append((st, pt, eng, u, ch, g, jt0))

            for k in range(NTOT + AV_LAG):
                # sims in quads: two groups' sims back-to-back (4 row groups)
                if k < NTOT and k % 2 == 0:
                    emit_sims(k)
                    if k + 1 < NTOT:
                        emit_sims(k + 1)
                if k >= EXP_LAG and k - EXP_LAG < NTOT:
                    emit_exp(recs[k - EXP_LAG])
                if k >= AV_LAG and k - AV_LAG < NTOT:
                    rec = recs[k - AV_LAG]
                    emit_avrs(rec)
                    recs[k - AV_LAG] = None
                    # chunk's last group -> schedule its epilogue next iter
                    if rec[6] == JT - 2:
                        epi_states.append((k + 1, rec[3], rec[4]))
                # run any due epilogue heads / fins
                for st_ in [e for e in epi_states if e[0] <= k]:
                    fin_states.append((k + FIN_LAG, emit_epilogue_head(
                        st_[1], st_[2])))
                    epi_states.remove(st_)
                for fs in [f for f in fin_states if f[0] <= k]:
                    emit_fin(fs[1])
                    fin_states.remove(fs)
            for st_ in epi_states:
                fin_states.append((0, emit_epilogue_head(st_[1], st_[2])))
            for fs in fin_states:
                emit_fin(fs[1])

        p12.close()

        # =========================== P3: output projection =================
        with ExitStack() as p3:
            sc3 = p3.enter_context(tc.tile_pool(name="p3scratch", bufs=2))
            py3 = p3.enter_context(tc.tile_pool(name="py3", bufs=4,
                                                space="PSUM"))
            engs = [nc.sync, nc.gpsimd, nc.scalar]
            for ch in range(NCHUNK):
                cs = slice(ch * CHUNK, (ch + 1) * CHUNK)
                for m in range(2):
                    ps = py3.tile([128, CHUNK], F32, name="psy", tag="psy")
                    for u in range(UNITS):
                        nc.tensor.matmul(
                            ps[:, :],
                            w_o[:, u, m, :],
                            oT[u][:, cs],
                            start=(u == 0), stop=(u == 1))
                    ysb = sc3.tile([128, CHUNK], F32, name="ysb", tag="ysb",
                                   bufs=4)
                    ceng = nc.vector if (ch + m) % 2 == 0 else nc.scalar
                    ceng.tensor_copy(ysb[:, :], ps[:, :]) \
                        if ceng is nc.vector else ceng.copy(ysb[:, :], ps[:, :])
                    engs[(2 * ch + m) % 3].dma_start(
                        out=y_out[m * 128:(m + 1) * 128, cs], in_=ysb[:, :])

    nc.compile()
    return nc


_NC_CACHE = None


def _get_nc():
    global _NC_CACHE
    if _NC_CACHE is None:
        _NC_CACHE = _build()
    return _NC_CACHE


def _make_in_maps(x, w_qkv, w_out):
    """Build the 8 per-core input dicts from full inputs."""
    x = np.ascontiguousarray(x, dtype=np.float32)
    w_qkv = np.ascontiguousarray(w_qkv, dtype=np.float32)
    w_out = np.ascontiguousarray(w_out, dtype=np.float32)
    b, c, h, w = x.shape
    xf = x.reshape(b, c, h * w)

    ones4 = np.zeros((128, 4), np.float16)
    for r in range(4):
        ones4[32 * r:32 * r + 32, r] = 1.0

    in_maps = []
    for core in range(NCORES):
        bb = core // 2
        p = core % 2
        heads = [2 * p, 2 * p + 1]
        wq = np.stack([w_qkv[hh * DIM_HEAD:(hh + 1) * DIM_HEAD, :]
                       for hh in heads])
        wk = np.stack([w_qkv[HIDDEN + hh * DIM_HEAD:
                             HIDDEN + (hh + 1) * DIM_HEAD, :] for hh in heads])
        wv = np.stack([w_qkv[2 * HIDDEN + hh * DIM_HEAD:
                             2 * HIDDEN + (hh + 1) * DIM_HEAD, :]
                       for hh in heads])
        wqT = np.ascontiguousarray(
            np.concatenate([np.transpose(wq, (0, 2, 1))] * 4, axis=2))
        wkT = np.ascontiguousarray(np.transpose(wk, (0, 2, 1)))
        wvT = np.ascontiguousarray(
            np.concatenate([wv[0].T, wv[1].T], axis=1))  # [256, 64]
        # wo97[u, d, m, mm] = w_out[m*128+mm, head_u*32+d] at rows 0-31 and
        # 64-95 (the two av stream partials); zero rows kill junk/rowsum rows.
        wo4 = np.zeros((UNITS, 97, 2, 128), np.float32)
        for u, hh in enumerate(heads):
            blk = w_out[:, hh * DIM_HEAD:(hh + 1) * DIM_HEAD].T.reshape(
                DIM_HEAD, 2, 128)
            wo4[u, 0:DIM_HEAD] = blk
            wo4[u, 64:64 + DIM_HEAD] = blk
        in_maps.append({
            "x_in": np.ascontiguousarray(xf[bb]).astype(np.float16),
            "wqT": wqT.astype(np.float16),
            "wkT": wkT.astype(np.float16),
            "wvT": wvT.astype(np.float16),
            "wo4": wo4.astype(np.float16),
            "ones4": ones4,
        })
    return in_maps


def kernel(x, w_qkv, w_out, b_out):
    nc = _get_nc()
    in_maps = _make_in_maps(x, w_qkv, w_out)
    res = run_bass_kernel_spmd(nc, in_maps, core_ids=list(range(NCORES)))
    outs = res.results
    y = np.zeros((B, C, N), np.float32)
    for bb in range(B):
        y[bb] = outs[2 * bb]["y_out"] + outs[2 * bb + 1]["y_out"]
    y += np.asarray(b_out, np.float32)[None, :, None]
    return y.reshape(B, C, H, W).astype(np.float32)
